# revision 1
# baseline (speedup 1.0000x reference)
"""Trainium2 Bass kernel: batched CRF forward algorithm (log partition).

Math (see reference): per sequence, forward scan over T=512 steps with
K=5 tags. transitions[START,:] = transitions[:,STOP] = -1e4, so in
exp-space the START row / STOP column of exp(transitions) are exact f32
zeros and only tags {0,1,2} carry state: K_eff = 3.

Exp-space recurrence per sequence (n, p in 0..2):
    a_1[n]   = exp(feat_0[n] + trans[n, START] - cbar)
    a_{t+1}[n] = sum_p W_t[n,p] * a_t[p],  W_t[n,p] = exp(feat_t[n] + trans[n,p] - cbar)
    alpha    = ln(sum_n exp(trans[STOP,n]) * a_T[n]) + sum(renorm logs) + T*cbar

cbar is a constant per-step log-growth estimate (host-derived from the
inputs); a periodic renormalization (every 32 steps) by the per-group
max keeps a in f32 range, with the logs of the maxes accumulated at the
end.

Distribution: pure data-parallel over the batch. Core c takes sequences
[c*1024, (c+1)*1024); on-chip layout is partition rho (128) x group g
(8) with seq = c*1024 + rho*8 + g. No collectives.

Engines: ScalarE (ACT) builds W = exp(feat + bias) chunks; VectorE runs
the sequential scan (broadcast-mul + segmented reduce per step) as TWO
independent interleaved chains of 4 groups each, which hides the
semaphore round-trip between dependent VectorE ops behind the other
chain's engine time; HWDGE DMA streams feats in t-chunks,
double-buffered. Cost-model timeline: ~227us per core (memory roofline
~29us; the kernel is VectorE-bound at ~460ns per scan step, dominated
by the fixed ~60-cycle-per-instruction DVE overhead on 1022 dependent
small ops).

build_program_pair is an explored alternative (GPSIMD builds pair
matrices W_{2t+1}@W_{2t} so VectorE scans half the steps); it is
correct but NOT faster: the 27-MAC pair products cost more bulk work
than the scan saves, and SWDGE descriptor generation runs on the
GPSIMD engine itself. Kept for reference; kernel() uses build_program.
"""
import numpy as np

import concourse.bass as bass
import concourse.bacc as bacc
import concourse.tile as tile
from concourse import mybir
from concourse.bass_utils import run_bass_kernel_spmd

F32 = mybir.dt.float32
EXP = mybir.ActivationFunctionType.Exp
LN = mybir.ActivationFunctionType.Ln
MUL = mybir.AluOpType.mult
ADD = mybir.AluOpType.add
MAX = mybir.AluOpType.max
AXX = mybir.AxisListType.X

P = 128          # partitions
NT = 3           # effective tags {0,1,2}
K = 5            # raw tags per timestep
NCORES = 8
START = 3
STOP = 4


def build_program(T=512, G=8, TC=64, RN=32, NS=2, repeats=1, hwdge=True):
    """Build the per-core Bass program (identical on all 8 cores).

    T: sequence length; G: batch groups per partition (B_core = 128*G);
    TC: timestep chunk size; RN: renorm cadence in steps.
    NS: number of independent interleaved scan chains (splits the G groups);
    with NS=2 the semaphore round-trip between dependent VectorE ops hides
    behind the other chain's engine time.
    """
    NCH = T // TC
    n_renorm = max(0, (T - 2 - RN) // RN + 1) if T - 1 >= RN else 0
    # renorms happen after steps t = RN, 2RN, ... while t <= T-32 guard below
    renorm_ts = [t for t in range(RN, T - 31, RN)]
    n_renorm = len(renorm_ts)

    nc = bacc.Bacc(
        "TRN2",
        target_bir_lowering=False,
        debug=False,
        enable_asserts=False,
        num_devices=NCORES,
    )
    feats = nc.dram_tensor("feats", [P * G, T * K], F32, kind="ExternalInput")
    aux = nc.dram_tensor("aux", [P, 16], F32, kind="ExternalInput")
    alpha = nc.dram_tensor("alpha", [P, G], F32, kind="ExternalOutput")

    fv = feats.ap().rearrange("(r g) (t k) -> r g t k", g=G, k=K)

    with tile.TileContext(nc) as tc:
        with (
            tc.tile_pool(name="auxp", bufs=1) as auxp,
            tc.tile_pool(name="rawp", bufs=2) as rawp,
            tc.tile_pool(name="st", bufs=1) as st,
        ):
            auxt = auxp.tile([P, 16], F32)
            nc.gpsimd.dma_start(out=auxt[:], in_=aux.ap())

            # Each instruction may carry at most ONE semaphore wait in this
            # walrus version. These absorber ops make each compute engine
            # observe the aux DMA early, so later ops never need a second
            # wait for it.
            act_scr = st.tile([P, 1], F32)
            dve_scr = st.tile([P, 1], F32)
            nc.scalar.copy(act_scr[:], auxt[:, 0:1])
            nc.vector.tensor_copy(dve_scr[:], auxt[:, 0:1])

            # W for the whole sequence stays resident in SBUF (147KB/part at
            # T=512); avoids pool slot-reuse waits on the ACT W-build ops.
            w_full = st.tile([P, G, T * 9], F32)
            w4 = w_full[:].rearrange("p g (t m) -> p g t m", m=9)

            assert G % NS == 0
            GH = G // NS  # groups per chain
            chains = []
            for h in range(NS):
                a = st.tile([P, GH * NT], F32, tag=f"a{h}")
                q = st.tile([P, GH * NT * NT], F32, tag=f"q{h}")
                mbuf = st.tile([P, max(n_renorm, 1) * GH], F32, tag=f"mb{h}")
                rinv = st.tile([P, GH], F32, tag=f"rv{h}")
                a3 = a[:].rearrange("p (g w) -> p g w", w=NT)
                q4 = q[:].rearrange("p (g n z) -> p g n z", n=NT, z=NT)
                a4 = a3.unsqueeze(2).broadcast_to((P, GH, NT, NT))
                chains.append(dict(a=a, q=q, mbuf=mbuf, rinv=rinv, a3=a3,
                                   q4=q4, a4=a4, g0=h * GH))

            def one_pass():
                r_i = 0
                for ch in range(NCH):
                    raw = rawp.tile([P, G, TC * K], F32)
                    raw4 = raw[:].rearrange("p g (t k) -> p g t k", k=K)
                    dmae = nc.sync if hwdge else nc.gpsimd
                    dmae.dma_start(
                        out=raw4, in_=fv[:, :, ch * TC : (ch + 1) * TC, :]
                    )
                    wc4 = w4[:, :, ch * TC : (ch + 1) * TC, :]
                    for n in range(NT):
                        rin = raw4[:, :, :, n]
                        for pp in range(NT):
                            j = 3 * n + pp
                            nc.scalar.activation(
                                wc4[:, :, :, j], rin, EXP, bias=auxt[:, j : j + 1]
                            )
                    if ch == 0:
                        # step 0: a_1[n] = exp(feat_0[n] + trans[n,START] - cbar)
                        for c in chains:
                            for n in range(NT):
                                nc.scalar.activation(
                                    c["a3"][:, :, n],
                                    raw4[:, c["g0"] : c["g0"] + GH, 0, n],
                                    EXP,
                                    bias=auxt[:, 9 + n : 10 + n],
                                )
                    t_lo = 1 if ch == 0 else 0
                    for tl in range(t_lo, TC):
                        t = ch * TC + tl
                        for c in chains:
                            wt = w4[:, c["g0"] : c["g0"] + GH, t, :].rearrange(
                                "p g (n z) -> p g n z", z=NT
                            )
                            nc.vector.tensor_tensor(c["q4"], c["a4"], wt, MUL)
                        for c in chains:
                            nc.vector.tensor_reduce(c["a3"], c["q4"], axis=AXX, op=ADD)
                        if t in renorm_ts:
                            for c in chains:
                                ms = c["mbuf"][:, r_i * GH : (r_i + 1) * GH]
                                nc.vector.tensor_reduce(ms, c["a3"], axis=AXX, op=MAX)
                            for c in chains:
                                ms = c["mbuf"][:, r_i * GH : (r_i + 1) * GH]
                                nc.vector.reciprocal(c["rinv"][:], ms)
                            for c in chains:
                                rb = c["rinv"][:].unsqueeze(2).broadcast_to((P, GH, NT))
                                nc.vector.tensor_tensor(c["a3"], c["a3"], rb, MUL)
                            r_i += 1
                assert r_i == n_renorm

                # terminal: s[g] = sum_n u[n] * a[g,n];  alpha = ln(s) + sum ln(m) + T*cbar
                s8 = st.tile([P, G], F32)
                for c in chains:
                    ub = auxt[:, 12:15].unsqueeze(1).broadcast_to((P, GH, NT))
                    q3 = c["q"][:, : GH * NT].rearrange("p (g w) -> p g w", w=NT)
                    nc.vector.tensor_tensor(q3, c["a3"], ub, MUL)
                    nc.vector.tensor_reduce(
                        s8[:, c["g0"] : c["g0"] + GH], q3, axis=AXX, op=ADD
                    )
                sl = st.tile([P, G], F32)
                nc.scalar.activation(sl[:], s8[:], LN)
                at = st.tile([P, G], F32)
                if n_renorm > 0:
                    msum = st.tile([P, G], F32)
                    for c in chains:
                        mlog = st.tile([P, n_renorm * GH], F32, tag=f"ml{c['g0']}")
                        nc.scalar.activation(mlog[:], c["mbuf"][:, : n_renorm * GH], LN)
                        nc.vector.tensor_reduce(
                            msum[:, c["g0"] : c["g0"] + GH],
                            mlog[:].rearrange("p (r g) -> p g r", g=GH),
                            axis=AXX,
                            op=ADD,
                        )
                    nc.vector.scalar_tensor_tensor(
                        at[:], sl[:], auxt[:, 15:16], msum[:], op0=ADD, op1=ADD
                    )
                else:
                    nc.vector.tensor_scalar_add(at[:], sl[:], auxt[:, 15:16])
                nc.gpsimd.dma_start(out=alpha.ap(), in_=at[:])
            for _rep in range(repeats):
                one_pass()
    nc.compile()
    return nc



def build_program_pair(T=512, G=8, TC=64, RNM=16, NS=2, repeats=1):
    """Pair-fused variant: GPSIMD builds P_j = W_{2j+1} @ W_{2j} (per-seq 3x3
    products, k-terms combined via DMA accumulate); VectorE scans T/2 macro
    steps. P slot 0 holds W_1 alone (step 0 is the closed-form init)."""
    assert T % TC == 0 and TC % 2 == 0
    NCH = T // TC
    TCP = TC // 2          # pairs per chunk
    NP = T // 2            # macro steps (slot 0 = W_1)
    renorm_js = [j for j in range(RNM, NP - 15, RNM)]
    n_renorm = len(renorm_js)
    S = G * TCP            # merged (g, tau) stream length per chunk

    nc = bacc.Bacc(
        "TRN2", target_bir_lowering=False, debug=False,
        enable_asserts=False, num_devices=NCORES,
    )
    feats = nc.dram_tensor("feats", [P * G, T * K], F32, kind="ExternalInput")
    aux = nc.dram_tensor("aux", [P, 16], F32, kind="ExternalInput")
    alpha = nc.dram_tensor("alpha", [P, G], F32, kind="ExternalOutput")
    fv = feats.ap().rearrange("(r g) (t k) -> r g t k", g=G, k=K)

    with tile.TileContext(nc) as tc:
        with (
            tc.tile_pool(name="auxp", bufs=1) as auxp,
            tc.tile_pool(name="rawp", bufs=2) as rawp,
            tc.tile_pool(name="wp", bufs=2) as wp,
            tc.tile_pool(name="tmpp", bufs=1) as tmpp,
            tc.tile_pool(name="st", bufs=1) as st,
        ):
            auxt = auxp.tile([P, 16], F32)
            nc.sync.dma_start(out=auxt[:], in_=aux.ap())
            act_scr = st.tile([P, 1], F32)
            dve_scr = st.tile([P, 1], F32)
            nc.scalar.copy(act_scr[:], auxt[:, 0:1])
            nc.vector.tensor_copy(dve_scr[:], auxt[:, 0:1])

            pbuf = st.tile([P, G, NP * 9], F32)
            pv4 = pbuf[:].rearrange("p g (j m) -> p g j m", m=9)

            assert G % NS == 0
            GH = G // NS
            chains = []
            for h in range(NS):
                a = st.tile([P, GH * NT], F32, tag=f"a{h}")
                q = st.tile([P, GH * NT * NT], F32, tag=f"q{h}")
                mbuf = st.tile([P, max(n_renorm, 1) * GH], F32, tag=f"mb{h}")
                rinv = st.tile([P, GH], F32, tag=f"rv{h}")
                a3 = a[:].rearrange("p (g w) -> p g w", w=NT)
                q4 = q[:].rearrange("p (g n z) -> p g n z", n=NT, z=NT)
                a4 = a3.unsqueeze(2).broadcast_to((P, GH, NT, NT))
                chains.append(dict(a=a, q=q, mbuf=mbuf, rinv=rinv, a3=a3,
                                   q4=q4, a4=a4, g0=h * GH))

            def one_pass():
                r_i = 0
                j_done = 0

                def scan_to(j_hi):
                    nonlocal r_i, j_done
                    for j in range(j_done, j_hi):
                        for c in chains:
                            wt = pv4[:, c["g0"] : c["g0"] + GH, j, :].rearrange(
                                "p g (n z) -> p g n z", z=NT
                            )
                            nc.vector.tensor_tensor(c["q4"], c["a4"], wt, MUL)
                        for c in chains:
                            nc.vector.tensor_reduce(c["a3"], c["q4"], axis=AXX, op=ADD)
                        if j in renorm_js:
                            for c in chains:
                                ms = c["mbuf"][:, r_i * GH : (r_i + 1) * GH]
                                nc.vector.tensor_reduce(ms, c["a3"], axis=AXX, op=MAX)
                            for c in chains:
                                ms = c["mbuf"][:, r_i * GH : (r_i + 1) * GH]
                                nc.vector.reciprocal(c["rinv"][:], ms)
                            for c in chains:
                                rb = c["rinv"][:].unsqueeze(2).broadcast_to(
                                    (P, GH, NT)
                                )
                                nc.vector.tensor_tensor(c["a3"], c["a3"], rb, MUL)
                            r_i += 1
                    j_done = j_hi

                for ch in range(NCH):
                    raw = rawp.tile([P, G, TC * K], F32)
                    raw4 = raw[:].rearrange("p g (t k) -> p g t k", k=K)
                    nc.sync.dma_start(
                        out=raw4, in_=fv[:, :, ch * TC : (ch + 1) * TC, :]
                    )
                    w = wp.tile([P, G, TC * 9], F32)
                    w4 = w[:].rearrange("p g (t m) -> p g t m", m=9)
                    for n in range(NT):
                        rin = raw4[:, :, :, n]
                        for pp in range(NT):
                            j = 3 * n + pp
                            nc.scalar.activation(
                                w4[:, :, :, j], rin, EXP, bias=auxt[:, j : j + 1]
                            )
                    if ch == 0:
                        for c in chains:
                            for n in range(NT):
                                nc.scalar.activation(
                                    c["a3"][:, :, n],
                                    raw4[:, c["g0"] : c["g0"] + GH, 0, n],
                                    EXP,
                                    bias=auxt[:, 9 + n : 10 + n],
                                )
                    # pair products: tmp_k[s, n, p] = W1[s, n, k] * W0[s, k, p]
                    wx = w[:].rearrange("p g (tau x m) -> p (g tau) x m", x=2, m=9)
                    W1nk = wx[:, :, 1, :].rearrange("p s (n k) -> p s n k", k=NT)
                    W0kp = wx[:, :, 0, :].rearrange("p s (n k) -> p s n k", k=NT)
                    tmps = []
                    for kk in range(NT):
                        tmp = tmpp.tile([P, S * 9], F32, tag=f"tm{kk}")
                        tmp4 = tmp[:].rearrange("p (s n z) -> p s n z", n=NT, z=NT)
                        in0 = W1nk[:, :, :, kk].unsqueeze(3).broadcast_to(
                            (P, S, NT, NT)
                        )
                        in1 = W0kp[:, :, kk, :].unsqueeze(2).broadcast_to(
                            (P, S, NT, NT)
                        )
                        nc.gpsimd.tensor_tensor(tmp4, in0, in1, MUL)
                        tmps.append(tmp)
                    j0 = ch * TCP
                    pc = pv4[:, :, j0 : j0 + TCP, :]
                    for kk, tmp in enumerate(tmps):
                        tv = tmp[:].rearrange(
                            "p (g tau m) -> p g tau m", tau=TCP, m=9
                        )
                        nc.sync.dma_start(
                            out=pc, in_=tv,
                            accum_op=(mybir.AluOpType.bypass if kk == 0 else ADD),
                        )
                    if ch == 0:
                        # overwrite garbage slot 0 with W_1 (macro step 0)
                        nc.sync.dma_start(
                            out=pv4[:, :, 0, :], in_=w4[:, :, 1, :]
                        )
                    # consume finished macro steps (previous chunk fully ready)
                    scan_to(ch * TCP)
                scan_to(NP)
                assert r_i == n_renorm

                s8 = st.tile([P, G], F32)
                for c in chains:
                    ub = auxt[:, 12:15].unsqueeze(1).broadcast_to((P, GH, NT))
                    q3 = c["q"][:, : GH * NT].rearrange("p (g w) -> p g w", w=NT)
                    nc.vector.tensor_tensor(q3, c["a3"], ub, MUL)
                    nc.vector.tensor_reduce(
                        s8[:, c["g0"] : c["g0"] + GH], q3, axis=AXX, op=ADD
                    )
                sl = st.tile([P, G], F32)
                nc.scalar.activation(sl[:], s8[:], LN)
                at = st.tile([P, G], F32)
                if n_renorm > 0:
                    msum = st.tile([P, G], F32)
                    for c in chains:
                        mlog = st.tile([P, n_renorm * GH], F32, tag=f"ml{c['g0']}")
                        nc.scalar.activation(
                            mlog[:], c["mbuf"][:, : n_renorm * GH], LN
                        )
                        nc.vector.tensor_reduce(
                            msum[:, c["g0"] : c["g0"] + GH],
                            mlog[:].rearrange("p (r g) -> p g r", g=GH),
                            axis=AXX,
                            op=ADD,
                        )
                    nc.vector.scalar_tensor_tensor(
                        at[:], sl[:], auxt[:, 15:16], msum[:], op0=ADD, op1=ADD
                    )
                else:
                    nc.vector.tensor_scalar_add(at[:], sl[:], auxt[:, 15:16])
                nc.sync.dma_start(out=alpha.ap(), in_=at[:])

            for _rep in range(repeats):
                one_pass()
    nc.compile()
    return nc


def make_aux(transitions, cbar, T):
    tr = np.asarray(transitions, np.float32)
    row = np.zeros(16, np.float32)
    row[0:9] = (tr[:NT, :NT] - cbar).reshape(9)
    row[9:12] = tr[:NT, START] - cbar
    row[12:15] = np.exp(tr[STOP, :NT])
    row[15] = T * cbar
    return np.ascontiguousarray(np.broadcast_to(row, (P, 16)))


def compute_cbar(feats, transitions):
    tr = np.asarray(transitions, np.float64)
    m = np.exp(tr[:NT, :NT])
    cbar = float(np.log(m.sum(1)).mean())
    cbar += float(np.asarray(feats[::257, :, :NT], np.float64).max(axis=-1).mean())
    return cbar


_prog = None


def kernel(feats, transitions):
    global _prog
    feats = np.ascontiguousarray(np.asarray(feats, np.float32))
    B, T, Kk = feats.shape
    assert (B, T, Kk) == (8192, 512, 5)
    if _prog is None:
        _prog = build_program(T=T)
    cbar = compute_cbar(feats, transitions)
    aux = make_aux(transitions, cbar, T)
    bc = B // NCORES
    fr = feats.reshape(NCORES, bc, T * Kk)
    in_maps = [{"feats": fr[c], "aux": aux} for c in range(NCORES)]
    res = run_bass_kernel_spmd(_prog, in_maps, core_ids=list(range(NCORES))).results
    out = np.concatenate(
        [np.asarray(res[c]["alpha"], np.float32).reshape(bc) for c in range(NCORES)]
    )
    return out



# revision 8
# speedup vs baseline: 5.2625x; 5.2625x over previous
"""Trainium2 Bass kernel: batched CRF forward algorithm (log partition).

Algorithm (tag-major, rank-1-seam chunked scan):

The reference recurrence (per sequence, K_eff = 3 live tags) is
    a(t+1) = D_t E a(t),   D_t = diag(exp(feat_t - cbar)),  E = exp(trans[:3,:3])
with a(1) = exp(feat_0 + trans[:,START] - cbar) and
    alpha = ln(u . a(T)) + T*cbar,  u = exp(trans[STOP,:3]).

Products of positive matrices contract to rank one (Birkhoff): the state
direction forgets its initial condition at ~0.3/step.  So the T=512 chain
is split into NC=32 chunks of L=16 steps; every chunk is scanned as a
VECTOR (not matrix) from the fixed start w=1 with V=6 warm-up steps of
real data before its window, making its in-window log-growth exact to
~1e-4.  All chunks of all sequences advance in lock-step, so each round
is just two big ops:

    PE : q = blockdiag(E) @ a        (fixed weights, SBUF->PSUM, bf16)
    DVE: a = q * d_round             (PSUM fp32 x SBUF fp32 -> SBUF bf16)

Layout: tags on partitions: partition = (slot s<42, tag n<3) = 126 rows;
columns = (chunk c, seq-in-slot q<25): 800 cols/round, split into 2
chains of 400 for latency hiding and the 512-col matmul limit.  Chunk 0
carries the exact initial condition (zero-padded warmup + injected init
at round V).  Seam stitching: after rounds V-1 and R-1 a fixed
ones-selector matmul + ACT ln reads per-chunk log-magnitudes; alpha is
assembled on-device from 2 strided reduces.  exp(feat-cbar) runs on ACT
from bf16 feats DMA'd in round-major order (5 pipelined blocks).

The host only reorders/casts inputs (free vs the HW timeline), computes
cbar, and concatenates per-core outputs.  Distribution: pure data
parallel, core k owns sequences [k*1024, (k+1)*1024).
"""
import numpy as np
import ml_dtypes

import concourse.bass as bass
import concourse.bacc as bacc
import concourse.tile as tile
from concourse import mybir
from concourse.bass_utils import run_bass_kernel_spmd

F32 = mybir.dt.float32
BF16 = mybir.dt.bfloat16
EXP = mybir.ActivationFunctionType.Exp
LN = mybir.ActivationFunctionType.Ln
MUL = mybir.AluOpType.mult
ADD = mybir.AluOpType.add
SUB = mybir.AluOpType.subtract
AXX = mybir.AxisListType.X
NPBF16 = ml_dtypes.bfloat16

NT = 3           # live tags {0,1,2}
K = 5            # raw tags per timestep
NCORES = 8
START = 3
STOP = 4

T = 512
L = 16           # chunk window length (steps)
V = 6            # warm-up steps per chunk
NC = T // L      # 32 chunks
R = L + V        # 22 rounds
SLOTS = 42
P = SLOTS * NT   # 126 used partitions
CQ = 25          # cols per chunk (42*25 = 1050 >= 1024 streams)
Q = NC * CQ      # 800 cols per round
CH = 2           # chains
QH = Q // CH     # 400 cols per chain
BC = 1024        # sequences per core

# round blocks for DMA/exp streaming (prefix sums must end at R)
RBLOCKS = [1, 2, 4, 7, 8]
assert sum(RBLOCKS) == R


def build_program(dbg=False):
    nc = bacc.Bacc(
        "TRN2",
        target_bir_lowering=False,
        debug=False,
        enable_asserts=False,
        num_devices=NCORES,
    )
    fraw = nc.dram_tensor("fraw", [P, R * Q], BF16, kind="ExternalInput")
    finit = nc.dram_tensor("finit", [P, CQ], BF16, kind="ExternalInput")
    aux = nc.dram_tensor("aux", [P, 8], F32, kind="ExternalInput")
    wE = nc.dram_tensor("wE", [P, P], BF16, kind="ExternalInput")
    wS = nc.dram_tensor("wS", [P, SLOTS], BF16, kind="ExternalInput")
    alpha = nc.dram_tensor("alpha", [SLOTS, CQ], F32, kind="ExternalOutput")
    if dbg:
        d_lnst = nc.dram_tensor("d_lnst", [SLOTS, Q], F32, kind="ExternalOutput")
        d_lnen = nc.dram_tensor("d_lnen", [SLOTS, Q], F32, kind="ExternalOutput")
        d_dall = nc.dram_tensor("d_dall", [P, R * Q], F32, kind="ExternalOutput")

    with tile.TileContext(nc) as tc:
        with (
            tc.tile_pool(name="cst", bufs=1) as cst,
            tc.tile_pool(name="rawp", bufs=2) as rawp,
            tc.tile_pool(name="dp", bufs=1) as dp,
            tc.tile_pool(name="st", bufs=1) as st,
            tc.psum_pool(name="qp", bufs=4) as qp,
            tc.psum_pool(name="sp", bufs=4) as sp,
        ):
            auxt = cst.tile([P, 8], F32)
            wEt = cst.tile([P, P], BF16)
            wSt = cst.tile([P, SLOTS], BF16)
            fin = cst.tile([P, CQ], BF16)
            nc.sync.dma_start(out=auxt[:], in_=aux.ap())
            nc.sync.dma_start(out=wEt[:], in_=wE.ap())
            nc.sync.dma_start(out=wSt[:], in_=wS.ap())
            nc.sync.dma_start(out=fin[:], in_=finit.ap())

            # absorbers: let each compute engine observe the const DMAs once
            # so later ops never need a second semaphore wait for them.
            scr_a = st.tile([P, 1], F32)
            scr_v = st.tile([P, 1], F32)
            nc.scalar.copy(scr_a[:], auxt[:, 0:1])
            nc.vector.tensor_copy(scr_v[:], auxt[:, 0:1])

            init_sb = st.tile([P, CQ], BF16)
            nc.scalar.activation(init_sb[:], fin[:], EXP, bias=auxt[:, 1:2])

            dall = dp.tile([P, R * Q], F32)
            a = [st.tile([P, QH], BF16, name=f"a{h}", tag=f"a{h}") for h in range(CH)]
            for h in range(CH):
                nc.vector.memset(a[h][:], 1.0)
            lnst = st.tile([SLOTS, Q], F32)
            lnen = st.tile([SLOTS, Q], F32)

            # streaming DMA + exp of the per-round d tables, ahead of rounds
            fview = fraw.ap()
            r0 = 0
            blk_start = {}
            for nb in RBLOCKS:
                raw = rawp.tile([P, nb * Q], BF16)
                nc.sync.dma_start(
                    out=raw[:], in_=fview[:, r0 * Q : (r0 + nb) * Q]
                )
                nc.scalar.activation(
                    dall[:, r0 * Q : (r0 + nb) * Q], raw[:], EXP,
                    bias=auxt[:, 0:1],
                )
                blk_start[r0] = True
                r0 += nb

            for r in range(R):
                if r in blk_start:
                    # absorber: block readiness becomes DVE program-order, so
                    # round muls never need a second semaphore wait.
                    nc.vector.tensor_copy(scr_v[:], dall[:, r * Q : r * Q + 1])
                for h in range(CH):
                    q = qp.tile([P, QH], F32)
                    nc.tensor.matmul(q[:], wEt[:], a[h][:], start=True, stop=True)
                    dsl = dall[:, r * Q + h * QH : r * Q + (h + 1) * QH]
                    nc.vector.tensor_tensor(a[h][:], q[:], dsl, MUL)
                    if r == V - 1:
                        s = sp.tile([SLOTS, QH], F32)
                        nc.tensor.matmul(s[:], wSt[:], a[h][:], start=True, stop=True)
                        nc.scalar.activation(
                            lnst[:, h * QH : (h + 1) * QH], s[:], LN
                        )
                    if r == V and h == 0:
                        # chunk 0: exact initial condition replaces the
                        # zeroed warm-up state
                        nc.vector.tensor_copy(a[0][:, 0:CQ], init_sb[:])
                    if r == R - 1:
                        s = sp.tile([SLOTS, QH], F32)
                        nc.tensor.matmul(s[:], wSt[:], a[h][:], start=True, stop=True)
                        nc.scalar.activation(
                            lnen[:, h * QH : (h + 1) * QH], s[:], LN
                        )

            # alpha[s, q] = lnen[c=NC-1] + sum_{c<NC-1} lnen[c] - sum_{c>=1} lnst[c]
            #              + T*cbar   (aux[:,2] = T*cbar)
            env = lnen[:].rearrange("p (c q) -> p q c", q=CQ)
            stv = lnst[:].rearrange("p (c q) -> p q c", q=CQ)
            es = st.tile([SLOTS, CQ], F32)
            ss = st.tile([SLOTS, CQ], F32)
            nc.vector.tensor_reduce(es[:], env[:, :, 0 : NC - 1], axis=AXX, op=ADD)
            nc.vector.tensor_reduce(ss[:], stv[:, :, 1:NC], axis=AXX, op=ADD)
            at = st.tile([SLOTS, CQ], F32)
            nc.vector.tensor_tensor(at[:], es[:], ss[:], SUB)
            nc.vector.scalar_tensor_tensor(
                at[:], env[:, :, NC - 1], auxt[0:SLOTS, 2:3], at[:],
                op0=ADD, op1=ADD,
            )
            nc.sync.dma_start(out=alpha.ap(), in_=at[:])
            if dbg:
                nc.sync.dma_start(out=d_lnst.ap(), in_=lnst[:])
                nc.sync.dma_start(out=d_lnen.ap(), in_=lnen[:])
                nc.sync.dma_start(out=d_dall.ap(), in_=dall[:])
    nc.compile()
    return nc


def compute_cbar(feats, transitions):
    tr = np.asarray(transitions, np.float64)
    m = np.exp(tr[:NT, :NT])
    cbar = float(np.log(m.sum(1)).mean())
    cbar += float(np.asarray(feats[::257, :, :NT], np.float64).max(axis=-1).mean())
    return cbar


def make_inputs(feats, transitions, cbar):
    """Host-side layout prep: returns per-core input dicts."""
    tr = np.asarray(transitions, np.float32)
    f3 = np.asarray(feats[:, :, :NT], np.float32)

    # per-(chunk, round) timestep, padded below 0
    t_ids = np.arange(NC)[:, None] * L - V + np.arange(R)[None, :]  # [NC, R]
    tclip = np.clip(t_ids, 0, T - 1)

    B = f3.shape[0]
    g = f3[:, tclip, :].astype(np.float32)            # [B, NC, R, 3]
    g[:, t_ids < 0, :] = -30.0                        # chunk-0 warmup: d ~ 0
    g[:, NC - 1, R - 1, :] += tr[STOP, :NT]           # u-fold into last round

    gp = g.reshape(NCORES, BC, NC, R, NT)
    pad = np.zeros((NCORES, SLOTS * CQ - BC, NC, R, NT), np.float32)
    gp = np.concatenate([gp, pad], axis=1)            # [8, 1050, NC, R, 3]
    # partition = (s, n); free = (r, c, q)
    fraw = np.ascontiguousarray(
        gp.reshape(NCORES, SLOTS, CQ, NC, R, NT)
        .transpose(0, 1, 5, 4, 3, 2)                  # core, s, n, r, c, q
        .reshape(NCORES, P, R * Q)
        .astype(NPBF16)
    )

    f0 = f3[:, 0, :]                                  # [B, 3]
    f0p = np.concatenate(
        [f0.reshape(NCORES, BC, NT),
         np.zeros((NCORES, SLOTS * CQ - BC, NT), np.float32)], axis=1
    )
    finit = np.ascontiguousarray(
        f0p.reshape(NCORES, SLOTS, CQ, NT)
        .transpose(0, 1, 3, 2)                        # core, s, n, q
        .reshape(NCORES, P, CQ)
        .astype(NPBF16)
    )

    aux = np.zeros((P, 8), np.float32)
    aux[:, 0] = -cbar
    bias_init = tr[:NT, START] - cbar                 # per tag n
    aux[:, 1] = np.tile(bias_init, SLOTS)
    aux[:, 2] = T * cbar

    E = np.exp(tr[:NT, :NT])                          # E[n, p]
    wE = np.zeros((P, P), np.float32)
    for s in range(SLOTS):
        wE[3 * s : 3 * s + 3, 3 * s : 3 * s + 3] = E.T   # lhsT[(s,p),(s,n)]
    wE = wE.astype(NPBF16)
    wS = np.zeros((P, SLOTS), np.float32)
    for s in range(SLOTS):
        wS[3 * s : 3 * s + 3, s] = 1.0
    wS = wS.astype(NPBF16)

    return [
        {
            "fraw": fraw[c],
            "finit": finit[c],
            "aux": aux,
            "wE": np.ascontiguousarray(wE),
            "wS": np.ascontiguousarray(wS),
        }
        for c in range(NCORES)
    ]


_prog = None


def kernel(feats, transitions):
    global _prog
    feats = np.ascontiguousarray(np.asarray(feats, np.float32))
    B, Tt, Kk = feats.shape
    assert (B, Tt, Kk) == (NCORES * BC, T, K)
    if _prog is None:
        _prog = build_program()
    cbar = compute_cbar(feats, transitions)
    in_maps = make_inputs(feats, transitions, cbar)
    res = run_bass_kernel_spmd(_prog, in_maps, core_ids=list(range(NCORES))).results
    out = np.concatenate(
        [
            np.asarray(res[c]["alpha"], np.float32).reshape(SLOTS * CQ)[:BC]
            for c in range(NCORES)
        ]
    )
    return out
